# revision 1
# baseline (speedup 1.0000x reference)
"""DigitCaps dynamic-routing kernel for Trainium2 (8 NeuronCores, Bass/Tile).

Math (per routing iteration, reformulated to avoid materializing u_hat):
    u_hat[b,i,j,u] = sum_k W[i,j,u,k] * x[b,k,i]
    s[b,ju]  = sum_{ki} X[ki,b] * (c[i,j] * W[ki,ju])          (PE matmul, K=9216)
    v        = squash(s)  with the reference's quirky j-axis norm
    G[ki,ju] = sum_b X[b,ki] * v[b,ju]                         (PE matmul, K=64)
    b[i,j]   = sum_{k,u} W[ki,ju] * G[ki,ju]                   (DVE STT w/ accum)
    b is AllReduced (sum) over the 8 cores each iteration (batch mean).

Sharding: data-parallel over batch B=512 -> 64 rows per core; W replicated.
Key perf choices (measured on HW):
  - fp32 s-chain: accumulation chains hide the fp32 LDWEIGHTS; bf16 with
    M=64 stationary defeats FWL and runs 3x slower.
  - bf16 G-matmuls: M=128 stationary enables fast weight load (68ns/MM vs
    533ns fp32); b-update averages 128*512 terms so bf16 noise is harmless.
  - G PSUM rounds are evacuated to SBUF by ACT in bank-halves so the next
    round's matmuls don't serialize behind DVE reads (PSUM bank hazard).
  - squash runs on DVE except Sqrt (ACT LUT-table reloads cost ~1.3us).
  - the per-iteration b AllReduce is split in two halves so the first
    collective overlaps the tail of the b-update compute.
"""

import sys

sys.path.insert(0, "/opt/trn_rl_repo")

from contextlib import ExitStack

import numpy as np

B = 512
NCORES = 8
BL = B // NCORES  # 64 local batch rows
K = 8             # in_units (primary capsule dim)
IC = 1152         # in_channels (number of primary capsules)
J = 10            # num_units (output capsules)
U = 16            # unit_size
JU = J * U        # 160
NT = IC // 128    # 9 i-chunks of 128
NKT = K * NT      # 72 ki-chunks of 128
BETA = 1.45
NUM_ROUTING = 3
NT_A = 5          # t2 chunks in the first AllReduce half
IC_A = NT_A * 128

_CACHE = {}


def _build_nc():
    import concourse.bass as bass
    import concourse.tile as tile
    from concourse import bacc, mybir
    from concourse.masks import make_identity

    f32 = mybir.dt.float32
    bf16 = mybir.dt.bfloat16
    Alu = mybir.AluOpType
    Act = mybir.ActivationFunctionType

    nc = bacc.Bacc("TRN2", target_bir_lowering=False, debug=False,
                   num_devices=NCORES)

    xs = nc.dram_tensor("xs", [BL, K, IC], f32, kind="ExternalInput").ap()
    w = nc.dram_tensor("w", [IC, J, U, K], f32, kind="ExternalInput").ap()
    out = nc.dram_tensor("out", [BL, J, 4, 4], f32, kind="ExternalOutput").ap()

    xs_flat = xs.rearrange("b k i -> b (k i)")          # [64, 9216]
    w_r = w.rearrange("(t p) j u k -> p t (j u k)", p=128)  # [128, 9, 1280]
    out_flat = out.rearrange("b j g h -> b (j g h)")    # [64, 160]

    with tile.TileContext(nc) as tc, ExitStack() as ctx:
        consts = ctx.enter_context(tc.tile_pool(name="consts", bufs=1))
        small = ctx.enter_context(tc.tile_pool(name="small", bufs=2))
        scratch = ctx.enter_context(tc.tile_pool(name="scratch", bufs=8))
        psum = ctx.enter_context(tc.tile_pool(name="psum", bufs=1, space="PSUM"))
        dram = ctx.enter_context(tc.tile_pool(name="dram", bufs=1, space="DRAM"))

        # ---- persistent SBUF tensors ----
        x2 = consts.tile([BL, K * IC], f32)          # x[b, (k i)]
        x2b = consts.tile([BL, K * IC], bf16)        # bf16 copy for G matmuls
        x1 = consts.tile([128, NKT, BL], f32)        # x^T per ki-chunk
        w_nat = consts.tile([128, NT, J * U * K], f32)  # W natural layout
        wp = consts.tile([128, NKT, JU], f32)        # c-scaled W (matmul rhs)
        crep = consts.tile([128, NT, JU], f32)       # c broadcast over u
        ident = consts.tile([BL, BL], f32)
        ones = consts.tile([128, 128], f32)

        # one PSUM tensor = all 8 banks; everything slices into it
        pall = psum.tile([128, K, 512], f32)

        # W as [p, t2, j, u, k] view for strided reads
        w5 = w_nat.rearrange("p t (j u k) -> p t j u k", j=J, u=U)

        # ---- loads: split into small pieces so transfers spread across the
        # 16 DMA engines (a single transfer lands on one engine @~22GB/s) ----
        HIC = IC // 2
        for k in range(K):
            for h in range(2):
                nc.sync.dma_start(
                    out=x2[:, k * IC + h * HIC:k * IC + (h + 1) * HIC],
                    in_=xs_flat[:, k * IC + h * HIC:k * IC + (h + 1) * HIC])
            if k + 1 < NT:
                t2 = k
                for h in range(2):
                    nc.sync.dma_start(out=w_nat[:, t2, h * 640:(h + 1) * 640],
                                      in_=w_r[:, t2, h * 640:(h + 1) * 640])
        for h in range(2):
            nc.sync.dma_start(out=w_nat[:, NT - 1, h * 640:(h + 1) * 640],
                              in_=w_r[:, NT - 1, h * 640:(h + 1) * 640])
        make_identity(nc, ident)
        nc.vector.memset(ones, 1.0)

        # bf16 cast of x for the G-pass (split ACT/DVE, overlaps the load)
        for k in range(K):
            sl = slice(k * IC, (k + 1) * IC)
            if k % 2 == 0:
                nc.scalar.copy(x2b[:, sl], x2[:, sl])
            else:
                nc.vector.tensor_copy(x2b[:, sl], x2[:, sl])

        # ---- build x1 = per-chunk transpose of x2 (PE transpose) ----
        # evacuation alternates ACT/DVE so neither engine paces the PE
        for t in range(NKT):
            ps = pall[:, t % K, :BL]
            nc.tensor.transpose(ps, x2[:, t * 128:(t + 1) * 128], ident)
            if t % 2 == 0:
                nc.scalar.copy(x1[:, t, :], ps)
            else:
                nc.vector.tensor_copy(x1[:, t, :], ps)

        bfulls = {}
        for it in range(NUM_ROUTING):
            if it > 0:
                # ---- softmax over i (given b_full from the AllReduce) ----
                bf_a, bf_b = bfulls[it - 1]
                expb = small.tile([128, NT, J], f32, name=f"expb{it}")
                # exp(b/B): fold the batch-mean 1/B into the exp scale;
                # split so the first half runs while AllReduce B drains
                nc.scalar.activation(
                    expb[:, :NT_A, :].rearrange("p t j -> p (t j)"),
                    bf_a.rearrange("p t j -> p (t j)"),
                    Act.Exp, scale=1.0 / B)
                nc.scalar.activation(
                    expb[:, NT_A:, :].rearrange("p t j -> p (t j)"),
                    bf_b.rearrange("p t j -> p (t j)"),
                    Act.Exp, scale=1.0 / B)
                # Z[j] = sum_i exp(b[i,j]), broadcast to 128 partitions via
                # an accumulating ones-matmul; bank 7 of PSUM
                zp = pall[:, K - 1, :J]
                for t2 in range(NT):
                    nc.tensor.matmul(zp, ones, expb[:, t2, :],
                                     start=(t2 == 0), stop=(t2 == NT - 1))
                zinv = small.tile([128, J], f32, name=f"zinv{it}")
                nc.vector.reciprocal(zinv, zp)
                # crep[i, (j,u)] = expb[i,j] * zinv[j]  (broadcast over u)
                for t2 in range(NT):
                    nc.vector.tensor_mul(
                        crep[:, t2, :].rearrange("p (j u) -> p j u", j=J),
                        expb[:, t2, :].unsqueeze(-1).broadcast_to([128, J, U]),
                        zinv.unsqueeze(-1).broadcast_to([128, J, U]))

            # ---- wp = crep * W on DVE (iters>0). Iteration 0 has uniform
            # c = 1/IC folded into the squash scales, so wp is just a
            # contiguous repack of the strided W view (split ACT/DVE). ----
            for t in range(NKT):
                k, t2 = divmod(t, NT)
                wp_v = wp[:, t, :].rearrange("p (j u) -> p j u", j=J)
                if it == 0:
                    # DVE only: ACT is already saturated in the load phase
                    # with the x2b casts and x1 evacuations
                    nc.vector.tensor_copy(wp_v, w5[:, t2, :, :, k])
                else:
                    nc.vector.tensor_mul(
                        wp_v, w5[:, t2, :, :, k],
                        crep[:, t2, :].rearrange("p (j u) -> p j u", j=J))

            # ---- s = X1^T @ wp : accumulate 72 chunks into PSUM bank 0 ----
            sp = pall[:BL, 0, :JU]
            for t in range(NKT):
                nc.tensor.matmul(sp, x1[:, t, :], wp[:, t, :],
                                 start=(t == 0), stop=(t == NKT - 1))

            # ---- squash (reference quirk: norm over the j axis per (b,u)) ----
            # ACT only does Sqrt here; everything else on DVE to avoid the
            # ~1.3us ACT LUT-table reload per function switch
            s_sb = small.tile([BL, JU], f32, name=f"s_sb{it}")
            nc.vector.tensor_copy(s_sb, sp)
            ssq = small.tile([BL, JU], f32, name=f"ssq{it}")
            nc.vector.tensor_mul(ssq, s_sb, s_sb)
            msq = small.tile([BL, U], f32, name=f"msq{it}")
            nc.vector.tensor_reduce(
                msq, ssq.rearrange("b (j u) -> b u j", j=J),
                axis=mybir.AxisListType.X, op=Alu.add)
            # iteration 0: s here is actually IC*s, so scale m by 1/IC^2 and
            # s by 1/IC while forming v
            sc2 = 1.0 / (IC * IC) if it == 0 else 1.0
            sc1 = 1.0 / IC if it == 0 else 1.0
            mag = small.tile([BL, U], f32, name=f"mag{it}")
            tpb = small.tile([BL, U], f32, name=f"tpb{it}")
            rin = small.tile([BL, U], f32, name=f"rin{it}")
            fv = small.tile([BL, U], f32, name=f"fv{it}")
            nc.scalar.activation(mag, msq, Act.Sqrt, scale=sc2)
            nc.vector.tensor_scalar(tpb, msq, sc2, BETA,
                                    op0=Alu.mult, op1=Alu.add)
            nc.vector.reciprocal(rin, tpb)
            nc.vector.tensor_mul(fv, mag, rin)
            v = small.tile([BL, JU], f32, name=f"v{it}")
            nc.vector.scalar_tensor_tensor(
                out=v.rearrange("b (j u) -> b j u", j=J),
                in0=s_sb.rearrange("b (j u) -> b j u", j=J),
                scalar=sc1,
                in1=fv.unsqueeze(1).broadcast_to([BL, J, U]),
                op0=Alu.mult, op1=Alu.mult)

            if it == NUM_ROUTING - 1:
                nc.sync.dma_start(out=out_flat, in_=v)
                continue
            vb = small.tile([BL, JU], bf16, name=f"vb{it}")
            nc.vector.tensor_copy(vb, v)

            # ---- G = X2^T-chunks @ v, per (t2): 8 banks; ACT evacuates in
            # bank-halves so the next round's matmuls overlap the DVE reads.
            # The b AllReduce is split: half A (t2 < NT_A) is sent as soon as
            # its STT accumulations finish, overlapping the rest of (d). ----
            b_part = small.tile([128, NT, J], f32, name=f"bpart{it}")
            cc_in_a = dram.tile([IC_A, J], f32, name=f"ccina{it}")
            cc_out_a = dram.tile([IC_A, J], f32, name=f"ccouta{it}",
                                 addr_space="Shared")
            cc_in_b = dram.tile([IC - IC_A, J], f32, name=f"ccinb{it}")
            cc_out_b = dram.tile([IC - IC_A, J], f32, name=f"ccoutb{it}",
                                 addr_space="Shared")
            for t2 in range(NT):
                g_sb = scratch.tile([128, K, JU], f32, name="g_sb", bufs=3)
                for h in range(2):
                    for k in range(h * 4, h * 4 + 4):
                        nc.tensor.matmul(
                            pall[:, k, :JU],
                            x2b[:, (k * NT + t2) * 128:
                                (k * NT + t2) * 128 + 128],
                            vb, start=True, stop=True)
                    nc.scalar.copy(g_sb[:, h * 4:h * 4 + 4, :],
                                   pall[:, h * 4:h * 4 + 4, :JU])
                g_fk = g_sb.rearrange("p k f -> p f k")
                for j in range(J):
                    so = scratch.tile([128, U, K], f32, name="stt_scratch")
                    nc.vector.scalar_tensor_tensor(
                        out=so,
                        in0=w5[:, t2, j, :, :],
                        scalar=1.0,
                        in1=g_fk[:, j * U:(j + 1) * U, :],
                        op0=Alu.mult, op1=Alu.mult,
                        accum_out=b_part[:, t2, j:j + 1])
                if t2 == NT_A - 1:
                    nc.sync.dma_start(
                        out=cc_in_a.rearrange("(t p) j -> p t j", p=128),
                        in_=b_part[:, :NT_A, :])
                    nc.gpsimd.collective_compute(
                        "AllReduce", Alu.add,
                        replica_groups=[list(range(NCORES))],
                        ins=[cc_in_a[:, :]], outs=[cc_out_a[:, :]])
            nc.sync.dma_start(
                out=cc_in_b.rearrange("(t p) j -> p t j", p=128),
                in_=b_part[:, NT_A:, :])
            nc.gpsimd.collective_compute(
                "AllReduce", Alu.add,
                replica_groups=[list(range(NCORES))],
                ins=[cc_in_b[:, :]], outs=[cc_out_b[:, :]])
            bf_a = small.tile([128, NT_A, J], f32, name=f"bfa{it}")
            bf_b = small.tile([128, NT - NT_A, J], f32, name=f"bfb{it}")
            nc.sync.dma_start(
                out=bf_a, in_=cc_out_a.rearrange("(t p) j -> p t j", p=128))
            nc.sync.dma_start(
                out=bf_b, in_=cc_out_b.rearrange("(t p) j -> p t j", p=128))
            bfulls[it] = (bf_a, bf_b)

    nc.compile()
    return nc


def _get_nc():
    if "nc" not in _CACHE:
        _CACHE["nc"] = _build_nc()
    return _CACHE["nc"]


def _run(x, W, trace=False, **kw):
    from concourse import bass_utils

    nc = _get_nc()
    x = np.ascontiguousarray(np.asarray(x, dtype=np.float32))
    W = np.ascontiguousarray(np.asarray(W, dtype=np.float32))
    in_maps = [
        {"xs": x[c * BL:(c + 1) * BL], "w": W}
        for c in range(NCORES)
    ]
    res = bass_utils.run_bass_kernel_spmd(
        nc, in_maps, core_ids=list(range(NCORES)), trace=trace, **kw)
    outs = [res.results[c]["out"] for c in range(NCORES)]
    full = np.concatenate(outs, axis=0).reshape(B, J, 4, U // 4)
    return full, res


def kernel(x, W):
    full, _ = _run(x, W, trace=False)
    return full



# revision 7
# speedup vs baseline: 1.3328x; 1.3328x over previous
"""DigitCaps dynamic-routing kernel for Trainium2 (8 NeuronCores, Bass/Tile).

i-sharded design (v2). Math per routing iteration:
    u_hat[b,i,j,u] = sum_k W[i,j,u,k] * x[b,k,i]
    c = softmax_i(b_ij);  s[b,ju] = sum_i c[i,j] u_hat[b,i,ju]
    v = squash(s)  (reference's quirky j-axis norm)
    b_ij[i,j] = (1/B) sum_{b,u} u_hat * v

Sharding: each core owns i-block M_c = [128c, 128c+128) plus a REPLICATED
tail block T = [1024, 1152) (scaled 1/8 where it would be 8x-counted).
Each core holds ALL 512 batch rows of its i-columns, so the b_ij update is
fully local. The only cross-core quantity is the i-sum of the unnormalized
s-partial: the softmax normalizer Z[j] = sum_i exp(b[i,j]) commutes out of
the i-sum, so each iteration needs ONE fused AllReduce of [s~ (512x160) ;
Z-partial (1x160)] in bf16. The final iteration uses a ReduceScatter with
an 8x-replicated Z row so each core receives exactly its 64 output rows.

Per-core work per iteration: 64 bf16 s-matmuls (M=128 -> FWL) + 64 bf16
G-matmuls + a mult+XY-reduce b-update, ~8x less DVE work than the
batch-sharded formulation. All inputs are pre-cast/pre-transposed to bf16
on the host, so no on-chip transposes or casts are needed.
"""

import sys

sys.path.insert(0, "/opt/trn_rl_repo")

from contextlib import ExitStack

import numpy as np

B = 512
NCORES = 8
BL = B // NCORES   # 64 output rows per core
K = 8              # in_units
IC = 1152          # in_channels
J = 10             # num_units
U = 16             # unit_size
JU = J * U         # 160
NBLK = 4           # batch blocks of 128
NH = 2             # 0 = main i-block (per-core), 1 = tail i-block (replicated)
NKT = NH * K       # 16 ki-chunks of 128
BETA = 1.45
NUM_ROUTING = 3

_CACHE = {}


def _build_nc():
    import concourse.bass as bass
    import concourse.tile as tile
    from concourse import bacc, mybir

    f32 = mybir.dt.float32
    bf16 = mybir.dt.bfloat16
    Alu = mybir.AluOpType
    Act = mybir.ActivationFunctionType

    nc = bacc.Bacc("TRN2", target_bir_lowering=False, debug=False,
                   num_devices=NCORES)

    # all inputs pre-cast to bf16 and pre-transposed on the host
    x2m = nc.dram_tensor("x2m", [B, K, 128], bf16, kind="ExternalInput").ap()
    x2t = nc.dram_tensor("x2t", [B, K, 128], bf16, kind="ExternalInput").ap()
    x1m = nc.dram_tensor("x1m", [128, K, B], bf16, kind="ExternalInput").ap()
    x1t = nc.dram_tensor("x1t", [128, K, B], bf16, kind="ExternalInput").ap()
    wm = nc.dram_tensor("wm", [128, J * U * K], bf16, kind="ExternalInput").ap()
    wt = nc.dram_tensor("wt", [128, J * U * K], bf16, kind="ExternalInput").ap()
    out = nc.dram_tensor("out", [BL, J, 4, 4], f32, kind="ExternalOutput").ap()

    out_flat = out.rearrange("b j g h -> b (j g h)")    # [64, 160]
    x2m_r = x2m.rearrange("(t p) k i -> p t k i", p=128)  # [128, 4, 8, 128]
    x2t_r = x2t.rearrange("(t p) k i -> p t k i", p=128)

    with tile.TileContext(nc) as tc, ExitStack() as ctx:
        consts = ctx.enter_context(tc.tile_pool(name="consts", bufs=1))
        small = ctx.enter_context(tc.tile_pool(name="small", bufs=2))
        scratch = ctx.enter_context(tc.tile_pool(name="scratch", bufs=8))
        psum = ctx.enter_context(tc.tile_pool(name="psum", bufs=1, space="PSUM"))
        dram = ctx.enter_context(tc.tile_pool(name="dram", bufs=1, space="DRAM"))

        # ---- persistent SBUF tensors ----
        x2b = consts.tile([128, NBLK, K, 256], bf16)   # x[b-part, blk, k, i']
        x1b = consts.tile([128, NH, K, B], bf16)       # x^T[i-part, h, k, b]
        w_natb = consts.tile([128, NH, J * U * K], bf16)
        w_pre = consts.tile([128, NH, K, JU], bf16)    # W[ki, ju]; tail x 1/8
        w_bupt = consts.tile([128, K, JU], bf16)       # unscaled tail W[ki, ju]
        wp = consts.tile([128, NH, K, JU], bf16)       # c~-scaled W (mm rhs)
        g_sb = consts.tile([128, NH, K, JU], bf16)     # G[ki, ju]
        bacc_sb = consts.tile([128, NH, J], f32)       # local b_ij
        onesC = consts.tile([128, 1], f32)
        ones8 = consts.tile([128, 1], f32)
        ones1 = consts.tile([1, 128], f32)

        # one PSUM tensor = all 8 banks
        pall = psum.tile([128, 8, 512], f32)

        nc.vector.memset(onesC, 1.0)
        nc.vector.memset(ones8, 1.0 / NCORES)
        nc.vector.memset(ones1, 1.0)

        # ---- warm up the collective stack with a tiny AllReduce that
        # overlaps the load phase (first collective pays ncfw setup) ----
        cc_w_in = dram.tile([128, 1], f32, name="ccwin")
        cc_w_out = dram.tile([128, 1], f32, name="ccwout", addr_space="Shared")
        nc.sync.dma_start(out=cc_w_in[:, :], in_=onesC)
        nc.gpsimd.collective_compute(
            "AllReduce", Alu.add,
            replica_groups=[list(range(NCORES))],
            ins=[cc_w_in[:, :]], outs=[cc_w_out[:, :]])

        # ---- loads: many small DMAs to spread across the DMA engines ----
        for h in range(NH):
            src = wm if h == 0 else wt
            for q in range(2):
                nc.sync.dma_start(out=w_natb[:, h, q * 640:(q + 1) * 640],
                                  in_=src[:, q * 640:(q + 1) * 640])
        for h in range(NH):
            src = x1m if h == 0 else x1t
            for k in range(K):
                nc.sync.dma_start(out=x1b[:, h, k, :], in_=src[:, k, :])
        for blk in range(NBLK):
            for k in range(K):
                nc.sync.dma_start(out=x2b[:, blk, k, 0:128],
                                  in_=x2m_r[:, blk, k, :])
                nc.sync.dma_start(out=x2b[:, blk, k, 128:256],
                                  in_=x2t_r[:, blk, k, :])

        # ---- one-time W repack to [ki, ju] layout (tail scaled 1/8 for
        # the s-chain; unscaled copy kept for the b-update) ----
        w5 = w_natb.rearrange("p h (j u k) -> p h j u k", j=J, u=U)
        for k in range(K):
            wpre_m = w_pre[:, 0, k, :].rearrange("p (j u) -> p j u", j=J)
            wpre_t = w_pre[:, 1, k, :].rearrange("p (j u) -> p j u", j=J)
            wbup_t = w_bupt[:, k, :].rearrange("p (j u) -> p j u", j=J)
            nc.scalar.copy(wpre_m, w5[:, 0, :, :, k])
            nc.vector.tensor_scalar_mul(wpre_t, w5[:, 1, :, :, k], 1.0 / NCORES)
            if k % 2 == 0:
                nc.scalar.copy(wbup_t, w5[:, 1, :, :, k])
            else:
                nc.vector.tensor_copy(wbup_t, w5[:, 1, :, :, k])

        for it in range(NUM_ROUTING):
            last = it == NUM_ROUTING - 1

            # ---- softmax numerator + Z partial (uniform c on iteration 0:
            # constants folded into the squash scales) ----
            if it > 0:
                expb = small.tile([128, NH, J], f32, name=f"expb{it}")
                nc.scalar.activation(
                    expb.rearrange("p h j -> p (h j)"),
                    bacc_sb.rearrange("p h j -> p (h j)"),
                    Act.Exp, scale=1.0 / B)
                # Z partial via partition-sum matmul; tail weighted 1/8
                zp = pall[0:1, 4, 0:J]
                nc.tensor.matmul(zp, onesC, expb[:, 0, :],
                                 start=True, stop=False)
                nc.tensor.matmul(zp, ones8, expb[:, 1, :],
                                 start=False, stop=True)
                zrow = small.tile([1, JU], bf16, name=f"zrow{it}")
                nc.vector.tensor_copy(
                    zrow.rearrange("p (j u) -> p j u", j=J),
                    zp.unsqueeze(-1).broadcast_to([1, J, U]))
                # wp = expb (bcast over u) * w_pre
                for t in range(NKT):
                    h, k = divmod(t, K)
                    nc.vector.scalar_tensor_tensor(
                        out=wp[:, h, k, :].rearrange("p (j u) -> p j u", j=J),
                        in0=w_pre[:, h, k, :].rearrange("p (j u) -> p j u", j=J),
                        scalar=1.0,
                        in1=expb[:, h, :].unsqueeze(-1).broadcast_to([128, J, U]),
                        op0=Alu.mult, op1=Alu.mult)
                rhs = wp
            else:
                rhs = w_pre

            # ---- s~ partial: accumulate 16 ki-chunks per batch block ----
            for blk in range(NBLK):
                for t in range(NKT):
                    h, k = divmod(t, K)
                    nc.tensor.matmul(
                        pall[:, blk, 0:JU],
                        x1b[:, h, k, blk * 128:(blk + 1) * 128],
                        rhs[:, h, k, :],
                        start=(t == 0), stop=(t == NKT - 1))

            # ---- evacuate + cast to bf16 and ship to the collective ----
            scc = small.tile([128, NBLK, JU], bf16, name=f"scc{it}")
            for blk in range(NBLK):
                if blk % 2 == 0:
                    nc.scalar.copy(scc[:, blk, :], pall[:, blk, 0:JU])
                else:
                    nc.vector.tensor_copy(scc[:, blk, :], pall[:, blk, 0:JU])

            if not last:
                nrows = B if it == 0 else B + 1
                cc_in = dram.tile([nrows, JU], bf16, name=f"ccin{it}")
                cc_out = dram.tile([nrows, JU], bf16, name=f"ccout{it}",
                                   addr_space="Shared")
                nc.sync.dma_start(
                    out=cc_in[0:B, :].rearrange("(t p) f -> p t f", p=128),
                    in_=scc)
                if it > 0:
                    nc.sync.dma_start(out=cc_in[B:B + 1, :], in_=zrow)
                nc.gpsimd.collective_compute(
                    "AllReduce", Alu.add,
                    replica_groups=[list(range(NCORES))],
                    ins=[cc_in[:, :]], outs=[cc_out[:, :]])
                sfull = small.tile([128, NBLK, JU], bf16, name=f"sfull{it}")
                nc.sync.dma_start(
                    out=sfull,
                    in_=cc_out[0:B, :].rearrange("(t p) f -> p t f", p=128))
                if it > 0:
                    zrowf = small.tile([1, JU], bf16, name=f"zrowf{it}")
                    nc.sync.dma_start(out=zrowf, in_=cc_out[B:B + 1, :])
            else:
                # final iteration: ReduceScatter with an 8x-replicated Z row
                # so core c receives exactly its 64 output rows + Z.
                cc_in = dram.tile([NCORES, BL + 1, JU], bf16, name=f"ccin{it}")
                cc_out = dram.tile([BL + 1, JU], bf16, name=f"ccout{it}")
                for c in range(NCORES):
                    blk, off = divmod(c * BL, 128)
                    nc.sync.dma_start(out=cc_in[c, 0:BL, :],
                                      in_=scc[off:off + BL, blk, :])
                    nc.sync.dma_start(out=cc_in[c, BL:BL + 1, :], in_=zrow)
                nc.gpsimd.collective_compute(
                    "ReduceScatter", Alu.add,
                    replica_groups=[list(range(NCORES))],
                    ins=[cc_in[:, :, :]], outs=[cc_out[:, :]])
                sfull = small.tile([BL, 1, JU], bf16, name=f"sfull{it}")
                nc.sync.dma_start(out=sfull[:, 0, :], in_=cc_out[0:BL, :])
                zrowf = small.tile([1, JU], bf16, name=f"zrowf{it}")
                nc.sync.dma_start(out=zrowf, in_=cc_out[BL:BL + 1, :])

            # ---- divide by Z (it>0) and squash ----
            NP = 128 if not last else BL
            NB = NBLK if not last else 1
            s_sb = small.tile([NP, NB, JU], f32, name=f"s_sb{it}")
            if it == 0:
                nc.vector.tensor_copy(s_sb, sfull)
                sc2 = 1.0 / (IC * IC)
                sc1 = 1.0 / IC
            else:
                zf = small.tile([1, JU], f32, name=f"zf{it}")
                nc.vector.tensor_copy(zf, zrowf)
                zi = small.tile([1, JU], f32, name=f"zi{it}")
                nc.vector.reciprocal(zi, zf)
                zb = pall[0:NP, 4, 0:JU]
                nc.tensor.matmul(zb, ones1[:, 0:NP], zi,
                                 start=True, stop=True)
                for blk in range(NB):
                    nc.vector.tensor_mul(s_sb[:, blk, :], sfull[:, blk, :], zb)
                sc2 = 1.0
                sc1 = 1.0
            ssq = small.tile([NP, NB, JU], f32, name=f"ssq{it}")
            nc.vector.tensor_mul(ssq, s_sb, s_sb)
            msq = small.tile([NP, NB, U], f32, name=f"msq{it}")
            nc.vector.tensor_reduce(
                msq, ssq.rearrange("p t (j u) -> p t u j", j=J),
                axis=mybir.AxisListType.X, op=Alu.add)
            mag = small.tile([NP, NB, U], f32, name=f"mag{it}")
            nc.scalar.activation(mag, msq, Act.Sqrt, scale=sc2)
            tpb = small.tile([NP, NB, U], f32, name=f"tpb{it}")
            nc.vector.tensor_scalar(tpb, msq, sc2, BETA,
                                    op0=Alu.mult, op1=Alu.add)
            rin = small.tile([NP, NB, U], f32, name=f"rin{it}")
            nc.vector.reciprocal(rin, tpb)
            fv = small.tile([NP, NB, U], f32, name=f"fv{it}")
            nc.vector.tensor_mul(fv, mag, rin)

            if last:
                v = small.tile([BL, JU], f32, name=f"v{it}")
                nc.vector.scalar_tensor_tensor(
                    out=v.rearrange("b (j u) -> b j u", j=J),
                    in0=s_sb[:, 0, :].rearrange("b (j u) -> b j u", j=J),
                    scalar=sc1,
                    in1=fv[:, 0, :].unsqueeze(1).broadcast_to([BL, J, U]),
                    op0=Alu.mult, op1=Alu.mult)
                nc.sync.dma_start(out=out_flat, in_=v)
                continue

            vb = small.tile([128, NBLK, JU], bf16, name=f"vb{it}")
            for blk in range(NBLK):
                nc.vector.scalar_tensor_tensor(
                    out=vb[:, blk, :].rearrange("p (j u) -> p j u", j=J),
                    in0=s_sb[:, blk, :].rearrange("p (j u) -> p j u", j=J),
                    scalar=sc1,
                    in1=fv[:, blk, :].unsqueeze(1).broadcast_to([128, J, U]),
                    op0=Alu.mult, op1=Alu.mult)

            # ---- G[ki, ju] = sum_b x[b, ki] v[b, ju] (contract all 512) ----
            for t in range(NKT):
                h, k = divmod(t, K)
                bank = t % 8
                for blk in range(NBLK):
                    nc.tensor.matmul(
                        pall[:, bank, 0:JU],
                        x2b[:, blk, k, h * 128:(h + 1) * 128],
                        vb[:, blk, :],
                        start=(blk == 0), stop=(blk == NBLK - 1))
                if t % 2 == 0:
                    nc.scalar.copy(g_sb[:, h, k, :], pall[:, bank, 0:JU])
                else:
                    nc.vector.tensor_copy(g_sb[:, h, k, :], pall[:, bank, 0:JU])

            # ---- b_ij = sum_{k,u} W[ki,ju] G[ki,ju] (mult + XY reduce) ----
            for h in range(NH):
                wsrc = w_pre[:, 0, :, :] if h == 0 else w_bupt
                prodf = scratch.tile([128, K * JU], f32, name="prodf", bufs=2)
                nc.vector.tensor_mul(
                    prodf, wsrc.rearrange("p k f -> p (k f)"),
                    g_sb[:, h, :, :].rearrange("p k f -> p (k f)"))
                nc.vector.tensor_reduce(
                    bacc_sb[:, h, :],
                    prodf.rearrange("p (k j u) -> p j k u", k=K, j=J),
                    axis=mybir.AxisListType.XY, op=Alu.add)

    nc.compile()
    return nc


def _get_nc():
    if "nc" not in _CACHE:
        _CACHE["nc"] = _build_nc()
    return _CACHE["nc"]


def _run(x, W, trace=False, **kw):
    import ml_dtypes
    from concourse import bass_utils

    bf = ml_dtypes.bfloat16
    nc = _get_nc()
    x = np.asarray(x, dtype=np.float32)
    W = np.asarray(W, dtype=np.float32)
    xb = x.astype(bf)                                   # [512, 8, 1152]
    xTb = x.transpose(2, 1, 0).astype(bf)               # [1152, 8, 512]
    wb = W.reshape(IC, J * U * K).astype(bf)            # [1152, 1280]
    x2t = np.ascontiguousarray(xb[:, :, 1024:])
    x1t = np.ascontiguousarray(xTb[1024:])
    wt = np.ascontiguousarray(wb[1024:])
    in_maps = [
        {
            "x2m": np.ascontiguousarray(xb[:, :, 128 * c:128 * (c + 1)]),
            "x2t": x2t,
            "x1m": np.ascontiguousarray(xTb[128 * c:128 * (c + 1)]),
            "x1t": x1t,
            "wm": np.ascontiguousarray(wb[128 * c:128 * (c + 1)]),
            "wt": wt,
        }
        for c in range(NCORES)
    ]
    res = bass_utils.run_bass_kernel_spmd(
        nc, in_maps, core_ids=list(range(NCORES)), trace=trace, **kw)
    outs = [res.results[c]["out"] for c in range(NCORES)]
    full = np.concatenate(outs, axis=0).reshape(B, J, 4, U // 4)
    return full, res


def kernel(x, W):
    full, _ = _run(x, W, trace=False)
    return full
